# revision 93
# baseline (speedup 1.0000x reference)
"""Trainium2 Bass kernel for the ELGCA block (dwconv3x3+gelu || conv1x1+gelu
-> pooled linear attention), data-parallel over batch on 8 NeuronCores.

Self-contained: hardcodes shapes B=16, C=128, H=W=128, f32.
kernel(**inputs) takes full unsharded inputs, returns the FULL output.

v2 design (vs DVE-heavy baseline):
  - dwconv3x3 runs on the PE as 9 accumulating diag-matmuls per 4-row group
    (lhsT = diag(w_tap) bf16, rhs = shifted window views of the bf16 input
    strip). Frees the DVE almost entirely.
  - bf16 matmuls (1 cyc/row) for dwconv / conv B / out2; conv A (q,k) stays
    fp32 end-to-end because qk feeds a near-saturated softmax. Inputs cast
    f32->bf16 in-flight by gpsimd (SWDGE) DMAs, bypassing the serialized
    HWDGE path.
  - conv1x1 A/B block-diag matmuls; single [128,512] gelu for v+l into a
    persistent bf16 vl_pack; l rows stored via casting DMA.
  - out2 = one block-diag [64,64] E matmul per 512-col chunk.
  - big batched DMAs (one per strip / 4-chunk group) with the (b,c)
    partition merge done by multi-dim DRAM APs; vertical pooling runs
    incrementally per chunk through a 16-row ring buffer.
Per-core layout (BPC=2 local images): partitions p=(b*64+c) everywhere.
NOTE: emission order defines the tile-framework dependency graph — every
tile must have its writer emitted before any reader (w9 ordering bug).
"""

import numpy as np
from contextlib import ExitStack

import concourse.bass as bass
import concourse.tile as tile
from concourse import bacc, mybir
from concourse import bass_utils
from concourse.masks import make_identity

F32 = mybir.dt.float32
BF16 = mybir.dt.bfloat16
AX = mybir.AxisListType
ALU = mybir.AluOpType
ACTF = mybir.ActivationFunctionType

N_CORES = 8
B_TOT, C, H, W = 16, 128, 128, 128
BPC = B_TOT // N_CORES          # 2 images per core
HW = H * W                      # 16384
C2 = C // 2                     # 64
C4 = C // 4                     # 32
WP = W + 2                      # padded row width for dwconv strips
RS = 32                         # x1 strip height (rows)
NSTRIP = H // RS                # 4
GR = 4                          # rows per dwconv psum group
GPS = RS // GR                  # 8 groups per strip
NCH = 512                       # conv1x1 / out2 column chunk
NCHUNKS = HW // NCH             # 32
LD = 4                          # chunks per x2 load / l / out2 store group
W2 = W // 2                     # 64
RPC = NCH // W                  # image rows per conv chunk (4)

# dwconv taps in row-major (dy, dx) order
TAPS = [(dy, dx) for dy in (-1, 0, 1) for dx in (-1, 0, 1)]


def build_nc(loops=1, dbg=False):
    nc = bacc.Bacc("TRN2", target_bir_lowering=False, debug=False,
                   num_devices=N_CORES)
    dbg_t = None
    if dbg:
        dbg_t = nc.dram_tensor("dbg", [128, 2 * 4096 + 64], F32,
                               kind="ExternalOutput").ap()
    x = nc.dram_tensor("x", [BPC, C, H, W], F32, kind="ExternalInput").ap()
    dw_w = nc.dram_tensor("dw_w", [C2, 1, 3, 3], F32, kind="ExternalInput").ap()
    dw_b = nc.dram_tensor("dw_b", [C2], F32, kind="ExternalInput").ap()
    qw = nc.dram_tensor("qkvl_w", [C, C2, 1, 1], F32, kind="ExternalInput").ap()
    qb = nc.dram_tensor("qkvl_b", [C], F32, kind="ExternalInput").ap()
    out = nc.dram_tensor("out", [BPC, C, H, W], F32, kind="ExternalOutput").ap()

    x1v = x[:, 0:C2, :, :]                                       # [2,64,H,W]
    x2f = x[:, C2:C, :, :].rearrange("b c h w -> b c (h w)")     # [2,64,HW]
    o_x1 = out[:, 0:C2, :, :]                                    # [2,64,H,W]
    o_l = out[:, C2:96, :, :].rearrange("b c h w -> b c (h w)")  # [2,32,HW]
    o_2 = out[:, 96:128, :, :].rearrange("b c h w -> b c (h w)")

    with tile.TileContext(nc) as tc, ExitStack() as ctx:
        consts = ctx.enter_context(tc.tile_pool(name="consts", bufs=1))
        xp = ctx.enter_context(tc.tile_pool(name="xp", bufs=4))
        oxp = ctx.enter_context(tc.tile_pool(name="oxp", bufs=2))
        rhp = ctx.enter_context(tc.tile_pool(name="rhp", bufs=3))
        qgp = ctx.enter_context(tc.tile_pool(name="qgp", bufs=2))
        vlp = ctx.enter_context(tc.tile_pool(name="vlp", bufs=1))
        rtp = ctx.enter_context(tc.tile_pool(name="rtp", bufs=1))
        pfp = ctx.enter_context(tc.tile_pool(name="pfp", bufs=1))
        trp = ctx.enter_context(tc.tile_pool(name="trp", bufs=1))
        atp = ctx.enter_context(tc.tile_pool(name="atp", bufs=1))
        o2st = ctx.enter_context(tc.tile_pool(name="o2st", bufs=2))
        ps_conv = ctx.enter_context(
            tc.tile_pool(name="ps_conv", bufs=2, space="PSUM"))
        ps_dw = ctx.enter_context(
            tc.tile_pool(name="ps_dw", bufs=2, space="PSUM"))
        ps_sm = ctx.enter_context(
            tc.tile_pool(name="ps_sm", bufs=2, space="PSUM"))
        ps_o2 = ctx.enter_context(
            tc.tile_pool(name="ps_o2", bufs=1, space="PSUM"))
        ps_tr = ctx.enter_context(
            tc.tile_pool(name="ps_tr", bufs=1, space="PSUM"))

        # ---------------- constants ----------------
        # Order matters three ways: (a) the sync (SP) HWDGE queue stays free
        # for the first rhs loads; (b) small const transfers (w9/qwS) go
        # before big ones so the DVE-side const builds aren't starved;
        # (c) on the in-order DVE queue, the lhsA build precedes the wd
        # (dw-diag) build because conv A is the earliest PE consumer.
        # conv weights first, on the otherwise-empty sync queue: their tiny
        # transfers must precede the big rhs/strip prefetches on the shared
        # DMA engines, since lhsA gates the very first PE work
        qwT = qw.rearrange("o i kh kw -> (i kh kw) o")   # [64ic, 128oc] view
        qwS = consts.tile([128, 128], F32)
        nc.scalar.dma_start(qwS[0:C2, :], qwT)
        nc.scalar.dma_start(qwS[C2:128, :], qwT)

        id_f32 = consts.tile([128, 128], F32)
        make_identity(nc, id_f32[:])

        w9 = consts.tile([128, 9], F32)              # dw weights per (b,c)
        dw9 = dw_w.rearrange("c o kh kw -> c (o kh kw)")
        dwb = consts.tile([128, 1], F32)
        dwb2 = dw_b.unsqueeze(1)
        qb2 = qb.unsqueeze(1)
        biasA = consts.tile([128, 1], F32)
        biasB = consts.tile([128, 1], F32)
        # w9 MUST be loaded before the wd build below is emitted: emission
        # order defines the dependency graph, and a read-before-write is
        # undefined on hardware (this was a real nondeterministic x1 bug)
        nc.scalar.dma_start(w9[0:C2, :], dw9)
        nc.scalar.dma_start(w9[C2:128, :], dw9)
        nc.scalar.dma_start(dwb[0:C2, :], dwb2)
        nc.scalar.dma_start(dwb[C2:128, :], dwb2)

        def emit_small_consts():
            # emitted on the sync queue after the first rhs load: keeps their
            # tiny transfers from stalling behind the big prefetches
            nc.sync.dma_start(biasA[0:C4, :], qb2[0:C4])
            nc.sync.dma_start(biasA[C4:C2, :], qb2[0:C4])
            nc.sync.dma_start(biasA[C2:96, :], qb2[C4:C2])
            nc.sync.dma_start(biasA[96:128, :], qb2[C4:C2])
            nc.sync.dma_start(biasB[0:C4, :], qb2[C2:96])
            nc.sync.dma_start(biasB[C4:C2, :], qb2[C2:96])
            nc.sync.dma_start(biasB[C2:96, :], qb2[96:128])
            nc.sync.dma_start(biasB[96:128, :], qb2[96:128])

        # conv1x1 weights, block-diagonal with batch-contiguous head layout:
        # A out rows = [q(b0) 0:32 | q(b1) 32:64 | k(b0) 64:96 | k(b1) 96:128]
        # B out rows = [v(b0) 0:32 | v(b1) 32:64 | l(b0) 64:96 | l(b1) 96:128]
        # A (q,k) stays fp32 end-to-end: qk feeds a near-saturated softmax
        # (|qk| ~ 900), so bf16's 4e-3 relative error becomes +-1.5 in the
        # exponent and corrupts the attention. B (v,l) tolerates bf16.
        lhsA = consts.tile([128, 128], F32)
        lafB = consts.tile([128, 128], F32)
        nc.vector.memset(lhsA[:], 0.0)
        nc.vector.memset(lafB[:], 0.0)
        nc.vector.tensor_copy(lhsA[0:C2, 0:C4], qwS[0:C2, 0:C4])
        nc.vector.tensor_copy(lhsA[0:C2, C2:96], qwS[0:C2, C4:C2])
        nc.vector.tensor_copy(lafB[0:C2, 0:C4], qwS[0:C2, C2:96])
        nc.vector.tensor_copy(lafB[0:C2, C2:96], qwS[0:C2, 96:128])
        nc.vector.tensor_copy(lhsA[C2:128, C4:C2], qwS[C2:128, 0:C4])
        nc.vector.tensor_copy(lhsA[C2:128, 96:128], qwS[C2:128, C4:C2])
        nc.vector.tensor_copy(lafB[C2:128, C4:C2], qwS[C2:128, C2:96])
        nc.vector.tensor_copy(lafB[C2:128, 96:128], qwS[C2:128, 96:128])
        lhsB = consts.tile([128, 128], BF16)
        nc.vector.tensor_copy(lhsB[:], lafB[:])

        idb = consts.tile([128, 128], BF16)
        nc.vector.tensor_copy(idb[:], id_f32[:])
        # 9 per-tap diagonal weight matrices (bf16), side by side
        wd = consts.tile([128, 9 * 128], BF16)
        for t in range(9):
            nc.vector.tensor_scalar_mul(wd[:, t * 128:(t + 1) * 128],
                                        idb[:], w9[:, t:t + 1])

        def one_pass():
            # vl_pack: [v(b0) 0:32 | v(b1) 32:64 | l(b0) 64:96 | l(b1) 96:128]
            vl_pack = vlp.tile([128, HW], BF16, tag="vl")
            # rt: h-pooled [q sums 0:64 | k max 64:128], 8-row ring buffer
            rt = rtp.tile([128, 16 * W2], F32, tag="rt")
            # qf[0:64] = [qf(b0)|qf(b1)]; kf[64:128] = [kf(b0)|kf(b1)]
            qf = pfp.tile([C2, H * W2 // 2], F32, tag="qf")
            kf = pfp.tile([128, H * W2 // 2], F32, tag="kf")
            qfT = [trp.tile([128, NCHUNKS * C4], F32, tag=f"qfT{b}",
                            name=f"qfT{b}") for b in range(BPC)]
            kfT = [trp.tile([128, NCHUNKS * C4], F32, tag=f"kfT{b}",
                            name=f"kfT{b}") for b in range(BPC)]
            qkt_ps = [ps_sm.tile([C2, C4], F32, tag="sm", name=f"qkt{b}")
                      for b in range(BPC)]

            # ---------- x1 strip machinery (dwconv on PE) ----------
            dwst = {"g": 0, "xb": {}, "ox": None}

            def emit_strip_load(s):
                xb = xp.tile([128, (RS + 2) * WP], BF16, tag="xb",
                             name=f"xb{s}")
                xb3 = xb.rearrange("p (r w) -> p r w", w=WP)
                nc.gpsimd.memset(xb3[:, :, 0:1], 0.0)
                nc.gpsimd.memset(xb3[:, :, WP - 1:WP], 0.0)
                ys = max(RS * s - 1, 0)
                ye = min(RS * s + RS + 1, H)
                r0 = 0 if s > 0 else 1
                if s == 0:
                    nc.gpsimd.memset(xb3[:, 0:1, :], 0.0)
                if s == NSTRIP - 1:
                    nc.gpsimd.memset(xb3[:, RS + 1:RS + 2, :], 0.0)
                nr = ye - ys
                nc.gpsimd.dma_start(xb3[0:C2, r0:r0 + nr, 1:W + 1],
                                    x1v[0, :, ys:ye, :])
                nc.gpsimd.dma_start(xb3[C2:128, r0:r0 + nr, 1:W + 1],
                                    x1v[1, :, ys:ye, :])
                return xb3

            def pump_dw(n):
                for _ in range(n):
                    g = dwst["g"]
                    if g >= NSTRIP * GPS:
                        return
                    s, gg = g // GPS, g % GPS
                    if gg == 0:
                        if s == 0 and 0 not in dwst["xb"]:
                            dwst["xb"][0] = emit_strip_load(0)
                        if s + 1 < NSTRIP and s + 1 not in dwst["xb"]:
                            dwst["xb"][s + 1] = emit_strip_load(s + 1)
                    if gg % 4 == 0:          # half-strip output staging
                        dwst["ox"] = oxp.tile([128, 4 * GR * W], F32,
                                              tag="ox", name=f"ox{s}{gg}")
                    xb3 = dwst["xb"][s]
                    ps = ps_dw.tile([128, GR * W], F32, tag="dwps")
                    ps3 = ps.rearrange("p (r w) -> p r w", w=W)
                    r0 = GR * gg + 1
                    for t, (dy, dx) in enumerate(TAPS):
                        nc.tensor.matmul(
                            ps3[:], wd[:, t * 128:(t + 1) * 128],
                            xb3[:, r0 + dy:r0 + GR + dy, 1 + dx:1 + dx + W],
                            start=(t == 0), stop=(t == 8))
                    ox = dwst["ox"]
                    nc.scalar.activation(ox[:, (gg % 4) * NCH:
                                             (gg % 4 + 1) * NCH],
                                         ps[:], ACTF.Gelu, bias=dwb[:, 0:1])
                    if gg % 4 == 3:
                        ox3 = ox.rearrange("p (r w) -> p r w", w=W)
                        y0 = RS * s + (gg // 4) * 16
                        rows = slice(y0, y0 + 16)
                        nc.scalar.dma_start(o_x1[0, :, rows, :],
                                            ox3[0:C2])
                        nc.scalar.dma_start(o_x1[1, :, rows, :],
                                            ox3[C2:128])
                    dwst["g"] = g + 1

            # ---------- attention half (vertical pools + transposes) -------
            # incremental vertical pool over the rt ring: after chunk j
            # (image rows 4j..4j+4 in ring slots), finalize oy rows 2j..2j+2
            rq = rt[0:C2, :].rearrange(
                "p (h2 two w2) -> p h2 two w2", two=2, w2=W2)
            qf3 = qf.rearrange("p (h2 w2) -> p h2 w2", w2=W2)
            rk = rt[C2:128, :].rearrange(
                "p (h2 two w2) -> p h2 two w2", two=2, w2=W2)
            kf3 = kf[C2:128, :].rearrange("p (h2 w2) -> p h2 w2", w2=W2)

            def ring_row(r):
                sl = r % 16
                return rt[:, sl * W2:(sl + 1) * W2]

            def emit_vpool(j):
                m0 = 2 * j
                h0 = (j % 4) * 2          # ring row-pair base for this chunk
                nc.vector.tensor_add(qf3[:, m0:m0 + 2, :],
                                     rq[:, h0:h0 + 2, 0, :],
                                     rq[:, h0:h0 + 2, 1, :])
                if m0 >= 1:
                    nc.vector.tensor_add(qf3[:, m0:m0 + 1, :],
                                         qf3[:, m0:m0 + 1, :],
                                         ring_row(4 * j - 1)[0:C2]
                                         .unsqueeze(1))
                nc.vector.tensor_add(qf3[:, m0 + 1:m0 + 2, :],
                                     qf3[:, m0 + 1:m0 + 2, :],
                                     ring_row(4 * j + 1)[0:C2].unsqueeze(1))
                nc.vector.tensor_max(kf3[:, m0:m0 + 2, :],
                                     rk[:, h0:h0 + 2, 0, :],
                                     rk[:, h0:h0 + 2, 1, :])

            # k(b1) at base 96 (illegal matmul base) -> kf[32:64]; issued per
            # 16 pooled rows as soon as the vertical max has produced them
            def emit_kfcopy(po0):
                pcols = slice(po0 * W2, (po0 + 16) * W2)
                nc.gpsimd.dma_start(kf[C4:C2, pcols], kf[96:128, pcols])

            def emit_attn_half(h):
                for part in range(2):
                    po0 = h * C4 + part * 16
                    po1 = po0 + 16
                    for bi in range(BPC):
                        Pq = C4 * bi
                        Pk = C2 if bi == 0 else C4
                        for si, (srcT, dstT, Ps) in enumerate(
                                ((qf, qfT[bi], Pq), (kf, kfT[bi], Pk))):
                            ps = ps_tr.tile([128, 512], F32, tag="trps",
                                            name=f"tr{h}{bi}{si}{part}")
                            for jj in range(8):
                                j2 = h * 16 + part * 8 + jj
                                nc.tensor.transpose(
                                    ps[:, jj * C4:(jj + 1) * C4],
                                    srcT[Ps:Ps + C4,
                                         j2 * 128:(j2 + 1) * 128],
                                    id_f32[Ps:Ps + C4, Ps:Ps + C4])
                            nc.vector.tensor_copy(
                                dstT[:, (h * 16 + part * 8) * C4:
                                     (h * 16 + part * 8 + 8) * C4],
                                ps[:, 0:8 * C4])
                        for jj in range(8):
                            j2 = h * 16 + part * 8 + jj
                            nc.tensor.matmul(
                                qkt_ps[bi][0:C4, :],
                                kfT[bi][:, j2 * C4:(j2 + 1) * C4],
                                qfT[bi][:, j2 * C4:(j2 + 1) * C4],
                                start=(h == 0 and part == 0 and jj == 0),
                                stop=(h == 1 and part == 1 and jj == 7))

            # ---------- conv1x1 chunks, x1 groups interleaved ----------
            rhs_t, rhsb_t = {}, {}

            def load_rhs_group(g):
                if g * LD >= NCHUNKS:
                    return
                r = rhp.tile([128, LD * NCH], F32, tag="rhs", name=f"rhs{g}")
                nc.sync.dma_start(r[:], x2f[:, :, g * LD * NCH:
                                             (g + 1) * LD * NCH])
                rb = rhp.tile([128, LD * NCH], BF16, tag="rhsb",
                              name=f"rhsb{g}")
                nc.gpsimd.tensor_copy(rb[:], r[:])
                rhs_t[g], rhsb_t[g] = r, rb

            # prime: rhs first (its transfers gate the first conv matmuls);
            # strip loads follow at chunk 0
            load_rhs_group(0)
            emit_small_consts()
            load_rhs_group(1)
            for j in range(NCHUNKS):
                cols = bass.ts(j, NCH)
                if j % LD == 0:
                    load_rhs_group(j // LD + 2)
                if j == 0:
                    dwst["xb"][0] = emit_strip_load(0)
                    dwst["xb"][1] = emit_strip_load(1)
                rhs, rhsb = rhs_t[j // LD], rhsb_t[j // LD]
                rv = rhs[:, (j % LD) * NCH:(j % LD + 1) * NCH]
                rvb = rhsb[:, (j % LD) * NCH:(j % LD + 1) * NCH]

                Ap = ps_conv.tile([128, NCH], F32, tag="cps")
                nc.tensor.matmul(Ap[:], lhsA[:], rv, start=True, stop=True)
                Bp = ps_o2.tile([128, NCH], F32, tag="o2", name=f"bp{j}")
                nc.tensor.matmul(Bp[:], lhsB[:], rvb, start=True, stop=True)
                qg = qgp.tile([128, NCH], F32, tag="qg")
                nc.scalar.activation(qg[:], Ap[:], ACTF.Gelu,
                                     bias=biasA[:, 0:1])

                # fused horizontal pooling into rt (both batches per op)
                rbase = (j % 4) * RPC * W2
                rrows = rt[:, rbase:rbase + RPC * W2]
                Xq = qg[0:C2, :].rearrange(
                    "p (h w2 two) -> p h w2 two", h=RPC, two=2)
                r3 = rrows[0:C2, :].rearrange("p (h w2) -> p h w2", h=RPC)
                nc.vector.tensor_add(r3[:], Xq[:, :, :, 0], Xq[:, :, :, 1])
                nc.vector.tensor_add(r3[:, :, 1:W2], r3[:, :, 1:W2],
                                     Xq[:, :, 0:W2 - 1, 1])
                Xk = qg[C2:128, :].rearrange(
                    "p (h w2 two) -> p h w2 two", h=RPC, two=2)
                m3 = rrows[C2:128, :].rearrange("p (h w2) -> p h w2", h=RPC)
                nc.vector.tensor_max(m3[:], Xk[:, :, :, 0], Xk[:, :, :, 1])

                nc.scalar.activation(vl_pack[:, cols], Bp[:], ACTF.Gelu,
                                     bias=biasB[:, 0:1])
                if j % LD == LD - 1:
                    cols4 = slice((j - LD + 1) * NCH, (j + 1) * NCH)
                    nc.gpsimd.dma_start(o_l[:, :, cols4],
                                        vl_pack[C2:128, cols4])

                emit_vpool(j)
                if j in (7, 15, 23, 31):
                    emit_kfcopy((j - 7) * 2)

                if j % 4 != 0:
                    pump_dw(1)           # 24 groups during the conv phase
                if j == 15:
                    emit_attn_half(0)
                elif j == NCHUNKS - 1:
                    emit_attn_half(1)

            # ---------- attention stats + block-diag E ----------
            pump_dw(2)                   # PE work to cover the stats chain
            E = atp.tile([C2, C2], BF16, tag="E")
            nc.vector.memset(E[:], 0.0)
            for bi in range(BPC):
                qk_s = atp.tile([C4, C4], F32, tag="qks", bufs=2)
                nc.scalar.mul(qk_s[:], qkt_ps[bi][0:C4, :], 1.0 / 9.0)
                nmax = atp.tile([C4, 1], F32, tag="nmax", bufs=2)
                nc.vector.tensor_reduce(nmax[:], qk_s[:], axis=AX.X,
                                        op=ALU.max, negate=True)
                ET = atp.tile([C4, C4], F32, tag="ET", bufs=2)
                nc.scalar.activation(ET[:], qk_s[:], ACTF.Exp,
                                     bias=nmax[:, 0:1])
                ssum = atp.tile([C4, 1], F32, tag="ssum", bufs=2)
                nc.vector.reduce_sum(ssum[:], ET[:], axis=AX.X)
                rec = atp.tile([C4, 1], F32, tag="rec", bufs=2)
                nc.vector.reciprocal(rec[:], ssum[:])
                ETn = atp.tile([C4, C4], F32, tag="ETn", bufs=2)
                nc.vector.tensor_scalar_mul(ETn[:], ET[:], rec[:, 0:1])
                # transpose must land at psum partition 0 (hw constraint);
                # b1's block is then shifted to partitions 32:64 via a
                # regular identity matmul
                etp = ps_sm.tile([C4, C4], F32, tag="sm", name=f"etp{bi}")
                nc.tensor.transpose(etp[:], ETn[:], id_f32[0:C4, 0:C4])
                if bi == 0:
                    nc.vector.tensor_copy(E[0:C4, 0:C4], etp[:])
                else:
                    E1t = atp.tile([C4, C4], BF16, tag="E1t")
                    nc.vector.tensor_copy(E1t[:], etp[:])
                    esh = ps_sm.tile([C2, C4], F32, tag="sm", name="esh")
                    nc.tensor.matmul(esh[C4:C2, :], idb[0:C4, 0:C4],
                                     E1t[:], start=True, stop=True)
                    nc.vector.tensor_copy(E[C4:C2, C4:C2], esh[C4:C2, :])

            if dbg:
                nc.sync.dma_start(dbg_t[0:C2, 0:4096], qf[:])
                nc.sync.dma_start(dbg_t[C4:128, 4096:8192], kf[C4:128, :])
                nc.gpsimd.dma_start(dbg_t[0:C2, 8192:8256], E[:])

            # ---------- out2 chunks, remaining x1 groups interleaved -------
            st = None
            for j in range(NCHUNKS):
                cols = bass.ts(j, NCH)
                # alternate psum pools: the conv pool is idle in this phase,
                # so out2 effectively gets 4 banks of pipeline depth
                o2pool = ps_o2 if j % 2 == 0 else ps_conv
                o2ps = o2pool.tile([C2, NCH], F32,
                                   tag="o2" if j % 2 == 0 else "cps",
                                   name=f"o2ps{j}")
                nc.tensor.matmul(o2ps[:], E[:], vl_pack[0:C2, cols],
                                 start=True, stop=True)
                if j % 2 == 0:
                    st = o2st.tile([C2, 2 * NCH], F32, tag="st", bufs=3)
                stv = st[:, (j % 2) * NCH:(j % 2 + 1) * NCH]
                if j % 2 == 0:
                    nc.vector.tensor_copy(stv, o2ps[:])
                else:
                    nc.scalar.copy(stv, o2ps[:])
                if j % 2 == 1:
                    cols2 = slice((j - 1) * NCH, (j + 1) * NCH)
                    nc.sync.dma_start(o_2[:, :, cols2], st[:])
                if j % 2 == 0 and j < 16:
                    pump_dw(1)           # last 8 groups, early in the tail

            pump_dw(NSTRIP * GPS)        # safety: finish any leftovers

        for _ in range(loops):
            one_pass()

    nc.compile()
    return nc


_NC_CACHE = None


def _get_nc():
    global _NC_CACHE
    if _NC_CACHE is None:
        _NC_CACHE = build_nc()
    return _NC_CACHE


def kernel(x, dw_w, dw_b, qkvl_w, qkvl_b):
    x = np.ascontiguousarray(np.asarray(x, dtype=np.float32))
    shared = {
        "dw_w": np.ascontiguousarray(np.asarray(dw_w, dtype=np.float32)),
        "dw_b": np.ascontiguousarray(np.asarray(dw_b, dtype=np.float32)),
        "qkvl_w": np.ascontiguousarray(np.asarray(qkvl_w, dtype=np.float32)),
        "qkvl_b": np.ascontiguousarray(np.asarray(qkvl_b, dtype=np.float32)),
    }
    nc = _get_nc()
    in_maps = [
        {"x": x[c * BPC:(c + 1) * BPC], **shared} for c in range(N_CORES)
    ]
    res = bass_utils.run_bass_kernel_spmd(nc, in_maps,
                                          core_ids=list(range(N_CORES)))
    return np.concatenate([res.results[c]["out"] for c in range(N_CORES)],
                          axis=0)
